# revision 1
# baseline (speedup 1.0000x reference)
"""DeepReservoir (2-layer leaky ESN) Trainium2 kernel.

Reference computation (per layer):
    u = x @ K + b
    h_t = 0.1*h_{t-1} + 0.9*tanh(u_t + h_{t-1} @ W)
Layer 1 consumes layer 0's states; output = concat(s0, s1) on features.

Kernel strategy (data-parallel over batch, 8 cores x B=4 samples):
  - On-chip layout: features on partitions, (time, batch) on the free dim.
    z^T tile [512->4x128 part, B cols] = sum_k W[k,:]^T @ h^T[k-tile].
  - State stored scaled: htil = h / 0.9, so the leaky update is one fused
    DVE op htil_t = 0.1*htil_{t-1} + tanh(z_t); the 0.9 factors fold into
    the weights (W' = 0.9*W, K1' = 0.81*K1... see host prep) and the final
    0.9 rescale happens on the host.
  - Input projections computed per 64-step chunk as wide GEMMs; biases are
    added for free during PSUM->SBUF evacuation via ScalarE Identity+bias.
  - Matmul operands in fp16 (fast weight load); accumulation, u and z in
    fp32. Layer-1 runs one chunk behind layer 0, interleaved per step.
"""
import sys
import types

import numpy as np

B_TOT, T, I, U = 32, 1024, 64, 512
NCORES = 8
B = B_TOT // NCORES          # 4 samples per core
ALPHA = 0.9

_COMPILED = {}


# ---------------------------------------------------------------------------
# environment patches (inlined so kernel.py is self-contained)
# ---------------------------------------------------------------------------
def _apply_patches():
    import concourse.tile as tilemod
    from concourse.vector_clock import ScopedClock

    if not getattr(tilemod.TileContext, "_drain_patch_applied", False):
        def _drain_and_barrier(self, tick_clock, wait_clock):
            nc = self.nc
            drain_inst = nc.sync.drain()
            wait_clock.add_sem_waits(
                drain_inst.ins, ScopedClock({None: tick_clock.global_clock})
            )
            waits = list(drain_inst.ins.sync_info.on_wait)
            if len(waits) > 1:
                drain_inst.ins.sync_info.on_wait = waits[:1]
                for w in waits[1:]:
                    extra = nc.sync.drain()
                    si = extra.ins.sync_info
                    if si is None:
                        import bass_rust
                        extra.ins.sync_info = bass_rust.SyncInfo(
                            on_wait=[w], on_update=[]
                        )
                    else:
                        si.on_wait = [w]
            nc.all_engine_barrier()
            assert self.sems is not None
            popped = nc._tile_sem_poison_stack.pop()
            assert popped is self._sem_poison
            nc.clear_and_free_semaphores(list(self.sems.allocated().values()))
            nc.all_engine_barrier()

        tilemod.TileContext._drain_and_barrier = _drain_and_barrier
        tilemod.TileContext._drain_patch_applied = True

    import antenv
    if not hasattr(antenv, "axon_hooks"):
        mod = types.ModuleType("antenv.axon_hooks")
        mod._hook = None
        mod.set_axon_ntff_profile_hook = lambda h: setattr(mod, "_hook", h)
        mod.get_axon_ntff_profile_hook = lambda: mod._hook
        sys.modules["antenv.axon_hooks"] = mod
        antenv.axon_hooks = mod
        try:
            from trn_agent_boot.trn_boot import _ntff_profile_via_ctypes
            hook = _ntff_profile_via_ctypes("/opt/axon/libaxon_pjrt.so")
            if hook is not None:
                mod.set_axon_ntff_profile_hook(hook)
        except Exception:
            pass


def _split_sync_waits(nc, max_waits=1):
    """The public walrus rejects instructions with >2 sync-wait commands.
    Spread overflow waits onto same-engine NOPs inserted just before."""
    import concourse.mybir as mybir

    for f in nc.m.functions:
        for blk in f.blocks:
            insts = blk.instructions
            out = []
            changed = False
            for inst in insts:
                si = getattr(inst, "sync_info", None)
                waits = list(si.on_wait) if si is not None else []
                if len(waits) > max_waits:
                    changed = True
                    overflow = waits[:-max_waits]
                    si.on_wait = waits[-max_waits:]
                    for i in range(0, len(overflow), max_waits):
                        nop = mybir.InstNoOp(
                            name=nc.get_next_instruction_name(),
                            sync_info=mybir.SyncInfo(
                                on_wait=overflow[i:i + max_waits], on_update=[]
                            ),
                            bass_nofuse=True,
                            engine=inst.engine,
                        )
                        out.append(nop)
                out.append(inst)
            if changed:
                blk.instructions = out


# ---------------------------------------------------------------------------
# kernel builder
# ---------------------------------------------------------------------------
def build_nc(T=T, Tc=64):
    import concourse.bass as bass
    import concourse.tile as tile
    import concourse.mybir as mybir

    f32 = mybir.dt.float32
    f16 = mybir.dt.float16
    NC = T // Tc               # number of chunks
    S = 4 * B                  # free cols per step (4 U-tiles x B)

    nc = bass.Bass(trn_type="TRN2")

    xT_d = nc.declare_dram_parameter("xT", (I, T * B), f16, isOutput=False)
    k0_d = nc.declare_dram_parameter("k0", (I, U), f16, isOutput=False)
    w0_d = nc.declare_dram_parameter("w0", (128, 4 * U), f16, isOutput=False)
    k1_d = nc.declare_dram_parameter("k1", (128, 4 * U), f16, isOutput=False)
    w1_d = nc.declare_dram_parameter("w1", (128, 4 * U), f16, isOutput=False)
    b0_d = nc.declare_dram_parameter("b0", (128, 4), f32, isOutput=False)
    b1_d = nc.declare_dram_parameter("b1", (128, 4), f32, isOutput=False)
    h0_d = nc.declare_dram_parameter("h0T", (U, T * B), f16, isOutput=True)
    h1_d = nc.declare_dram_parameter("h1T", (U, T * B), f16, isOutput=True)

    with tile.TileContext(nc) as tc:
        import contextlib
        with contextlib.ExitStack() as ctx:
            const = ctx.enter_context(tc.tile_pool(name="const", bufs=1))
            hists = ctx.enter_context(tc.tile_pool(name="hists", bufs=1))
            upool = ctx.enter_context(tc.tile_pool(name="upool", bufs=1))
            zpool = ctx.enter_context(tc.tile_pool(name="zpool", bufs=3))
            ypool = ctx.enter_context(tc.tile_pool(name="ypool", bufs=3))
            ps_step = ctx.enter_context(
                tc.tile_pool(name="ps_step", bufs=3, space="PSUM"))
            ps_gemm = ctx.enter_context(
                tc.tile_pool(name="ps_gemm", bufs=2, space="PSUM"))

            # --- resident constants -----------------------------------------
            xT = const.tile([I, T * B], f16, tag="xT")
            nc.sync.dma_start(xT[:], xT_d[:, :])
            k0 = const.tile([I, U], f16, tag="k0")
            nc.sync.dma_start(k0[:], k0_d[:, :])
            w0 = const.tile([128, 4 * U], f16, tag="w0")
            nc.sync.dma_start(w0[:], w0_d[:, :])
            k1 = const.tile([128, 4 * U], f16, tag="k1")
            nc.sync.dma_start(k1[:], k1_d[:, :])
            w1 = const.tile([128, 4 * U], f16, tag="w1")
            nc.sync.dma_start(w1[:], w1_d[:, :])
            b0 = const.tile([128, 4], f32, tag="b0")
            nc.sync.dma_start(b0[:], b0_d[:, :])
            b1 = const.tile([128, 4], f32, tag="b1")
            nc.sync.dma_start(b1[:], b1_d[:, :])
            zero = const.tile([128, S], f16, tag="zero")
            nc.vector.memset(zero[:], 0.0)

            # double-buffered chunk tensors
            hist0 = [const.tile([128, Tc * S], f16, tag=f"hist0_{i}", name=f"hist0_{i}") for i in range(2)]
            hist1 = [const.tile([128, Tc * S], f16, tag=f"hist1_{i}", name=f"hist1_{i}") for i in range(2)]
            u0c = [const.tile([128, Tc * S], f32, tag=f"u0c_{i}", name=f"u0c_{i}") for i in range(2)]
            u1c = [const.tile([128, Tc * S], f32, tag=f"u1c_{i}", name=f"u1c_{i}") for i in range(2)]

            # chunk buffers use slab layout: col = j*(Tc*B) + t*B + b
            # (j = feature 128-tile). Slabs are contiguous for DMA and GEMMs;
            # per-step access is the strided [128, 4, B] cross-section.
            SLAB = Tc * B

            def mm_rhs_ap(hist_bufs, t, j):
                """[128, B] moving operand: state k-tile j at step t."""
                if t < 0:
                    return zero[:, j * B:(j + 1) * B]
                return hist_bufs[(t // Tc) % 2][
                    :, j * SLAB + (t % Tc) * B: j * SLAB + (t % Tc + 1) * B]

            def state_xs(hist_bufs, t):
                """[128, 4, B] strided cross-section of the state at step t."""
                if t < 0:
                    return zero[:, :].rearrange("p (j b) -> p j b", j=4)
                buf = hist_bufs[(t // Tc) % 2]
                r = t % Tc
                return buf[:, :].rearrange(
                    "p (j tb) -> p j tb", j=4)[:, :, r * B:(r + 1) * B]

            def u0_gemm(c):
                """u0 chunk c = x[chunk]^T-projected, bias added on evacuation."""
                dst = u0c[c % 2]
                cols = slice(c * Tc * B, (c + 1) * Tc * B)
                for m in range(4):
                    pg = ps_gemm.tile([128, Tc * B], mybir.dt.float32, tag="pg", name="pg")
                    nc.tensor.matmul(
                        pg[:], lhsT=k0[:, m * 128:(m + 1) * 128],
                        rhs=xT[:, cols], start=True, stop=True)
                    nc.scalar.activation(
                        dst[:, m * SLAB:(m + 1) * SLAB], pg[:],
                        mybir.ActivationFunctionType.Identity,
                        bias=b0[:, m:m + 1], scale=1.0)

            def u1_gemm(c):
                """u1 chunk c from hist0 chunk c (scales folded into k1)."""
                src = hist0[c % 2]
                dst = u1c[c % 2]
                for m in range(4):
                    pg = ps_gemm.tile([128, Tc * B], mybir.dt.float32, tag="pg", name="pg")
                    for k in range(4):
                        nc.tensor.matmul(
                            pg[:],
                            lhsT=k1[:, k * U + m * 128: k * U + (m + 1) * 128],
                            rhs=src[:, k * SLAB:(k + 1) * SLAB],
                            start=(k == 0), stop=(k == 3))
                    nc.scalar.activation(
                        dst[:, m * SLAB:(m + 1) * SLAB], pg[:],
                        mybir.ActivationFunctionType.Identity,
                        bias=b1[:, m:m + 1], scale=1.0)

            def step(layer, t):
                w, uc, hist = (
                    (w0, u0c, hist0) if layer == 0 else (w1, u1c, hist1))
                zb = ps_step.tile([128, S], mybir.dt.float32, tag="zb", name="zb")
                for m in range(4):
                    for k in range(4):
                        nc.tensor.matmul(
                            zb[:, m * B:(m + 1) * B],
                            lhsT=w[:, k * U + m * 128: k * U + (m + 1) * 128],
                            rhs=mm_rhs_ap(hist, t - 1, k),
                            start=(k == 0), stop=(k == 3))
                r = t % Tc
                u_ap = uc[(t // Tc) % 2][:, :].rearrange(
                    "p (m tb) -> p m tb", m=4)[:, :, r * B:(r + 1) * B]
                z = zpool.tile([128, S], mybir.dt.float32, tag="z", name="z")
                nc.vector.tensor_add(
                    z[:].rearrange("p (m b) -> p m b", m=4), u_ap,
                    zb[:].rearrange("p (m b) -> p m b", m=4))
                y = ypool.tile([128, S], f16, tag="y", name="y")
                nc.scalar.activation(
                    y[:], z[:], mybir.ActivationFunctionType.Tanh)
                nc.vector.scalar_tensor_tensor(
                    state_xs(hist, t), state_xs(hist, t - 1), 0.1,
                    y[:].rearrange("p (j b) -> p j b", j=4),
                    op0=mybir.AluOpType.mult, op1=mybir.AluOpType.add)

            def dma_out(hist_bufs, dram, c):
                src = hist_bufs[c % 2]
                cols = slice(c * Tc * B, (c + 1) * Tc * B)
                for j in range(4):
                    nc.sync.dma_start(
                        dram[j * 128:(j + 1) * 128, cols],
                        src[:, j * SLAB:(j + 1) * SLAB])

            u0_gemm(0)
            for c in range(NC + 1):
                for r in range(Tc):
                    if c < NC:
                        step(0, c * Tc + r)
                    if c >= 1:
                        step(1, (c - 1) * Tc + r)
                if c < NC:
                    u1_gemm(c)
                    dma_out(hist0, h0_d, c)
                    if c + 1 < NC:
                        u0_gemm(c + 1)
                if c >= 1:
                    dma_out(hist1, h1_d, c - 1)

    _split_sync_waits(nc)
    return nc


# ---------------------------------------------------------------------------
# host wrapper
# ---------------------------------------------------------------------------
def _prep_weight(w, scale):
    """[U,U] -> [128, 4*U] fp16 with block (k,m) at cols k*U + m*128."""
    a = (scale * w).astype(np.float16)
    return np.ascontiguousarray(
        a.reshape(4, 128, 4, 128).transpose(1, 0, 2, 3).reshape(128, 4 * U))


def kernel(x, kernel0, rec0, bias0, kernel1, rec1, bias1):
    _apply_patches()
    from concourse.bass_utils import run_bass_kernel_spmd

    x = np.asarray(x, dtype=np.float32)
    kernel0 = np.asarray(kernel0, dtype=np.float32)
    rec0 = np.asarray(rec0, dtype=np.float32)
    bias0 = np.asarray(bias0, dtype=np.float32)
    kernel1 = np.asarray(kernel1, dtype=np.float32)
    rec1 = np.asarray(rec1, dtype=np.float32)
    bias1 = np.asarray(bias1, dtype=np.float32)

    if "nc" not in _COMPILED:
        _COMPILED["nc"] = build_nc()
    nc = _COMPILED["nc"]

    in_maps = _make_in_maps(x, kernel0, rec0, bias0, kernel1, rec1, bias1)
    res = run_bass_kernel_spmd(nc, in_maps, list(range(NCORES)))

    out = np.empty((B_TOT, T, 2 * U), dtype=np.float32)
    for c in range(NCORES):
        for name, off in (("h0T", 0), ("h1T", U)):
            h = res.results[c][name].astype(np.float32) * ALPHA  # [U, T*B]
            out[c * B:(c + 1) * B, :, off:off + U] = (
                h.reshape(U, T, B).transpose(2, 1, 0))
    return out


def _make_in_maps(x, kernel0, rec0, bias0, kernel1, rec1, bias1):
    k0 = kernel0.astype(np.float16)
    w0 = _prep_weight(rec0, ALPHA)
    k1 = _prep_weight(kernel1, ALPHA)
    w1 = _prep_weight(rec1, ALPHA)
    b0 = np.ascontiguousarray(bias0.reshape(4, 128).T).astype(np.float32)
    b1 = np.ascontiguousarray(bias1.reshape(4, 128).T).astype(np.float32)
    in_maps = []
    for c in range(NCORES):
        xc = x[c * B:(c + 1) * B]
        xT = np.ascontiguousarray(
            xc.transpose(2, 1, 0).reshape(I, T * B)).astype(np.float16)
        in_maps.append({
            "xT": xT, "k0": k0, "w0": w0, "k1": k1, "w1": w1,
            "b0": b0, "b1": b1,
        })
    return in_maps


def run_timed(x, kernel0, rec0, bias0, kernel1, rec1, bias1, tmpdir=None):
    """Run with NTFF profiling; returns BassKernelResults with exec_time_ns."""
    _apply_patches()
    import tempfile
    if tmpdir is None:
        tmpdir = tempfile.mkdtemp(prefix="dr_trace_")
    from concourse.bass_utils import run_bass_kernel_spmd
    if "nc" not in _COMPILED:
        _COMPILED["nc"] = build_nc()
    in_maps = _make_in_maps(
        np.asarray(x, np.float32), np.asarray(kernel0, np.float32),
        np.asarray(rec0, np.float32), np.asarray(bias0, np.float32),
        np.asarray(kernel1, np.float32), np.asarray(rec1, np.float32),
        np.asarray(bias1, np.float32))
    return run_bass_kernel_spmd(
        _COMPILED["nc"], in_maps, list(range(NCORES)), trace=True,
        tmpdir=tmpdir)



# revision 7
# speedup vs baseline: 1.2259x; 1.2259x over previous
"""DeepReservoir (2-layer leaky ESN) Trainium2 kernel.

Reference computation (per layer):
    u = x @ K + b
    h_t = 0.1*h_{t-1} + 0.9*tanh(u_t + h_{t-1} @ W)
Layer 1 consumes layer 0's states; output = concat(s0, s1) on features.

Kernel strategy (data-parallel over batch, 8 cores x B=4 samples):
  - On-chip layout: features on partitions, (time, batch) on the free dim.
    z^T tile [512->4x128 part, B cols] = sum_k W[k,:]^T @ h^T[k-tile].
  - State stored scaled: htil = h / 0.9, so the leaky update is one fused
    DVE op htil_t = 0.1*htil_{t-1} + tanh(z_t); the 0.9 factors fold into
    the weights (W' = 0.9*W, K1' = 0.81*K1... see host prep) and the final
    0.9 rescale happens on the host.
  - Input projections computed per 64-step chunk as wide GEMMs; biases are
    added for free during PSUM->SBUF evacuation via ScalarE Identity+bias.
  - Matmul operands in fp16 (fast weight load); accumulation, u and z in
    fp32. Layer-1 runs one chunk behind layer 0, interleaved per step.
"""
import sys
import types

import numpy as np

B_TOT, T, I, U = 32, 1024, 64, 512
NCORES = 8
B = B_TOT // NCORES          # 4 samples per core
ALPHA = 0.9

_COMPILED = {}


# ---------------------------------------------------------------------------
# environment patches (inlined so kernel.py is self-contained)
# ---------------------------------------------------------------------------
def _apply_patches():
    import concourse.tile as tilemod
    from concourse.vector_clock import ScopedClock

    if not getattr(tilemod.TileContext, "_drain_patch_applied", False):
        def _drain_and_barrier(self, tick_clock, wait_clock):
            nc = self.nc
            drain_inst = nc.sync.drain()
            wait_clock.add_sem_waits(
                drain_inst.ins, ScopedClock({None: tick_clock.global_clock})
            )
            waits = list(drain_inst.ins.sync_info.on_wait)
            if len(waits) > 1:
                drain_inst.ins.sync_info.on_wait = waits[:1]
                for w in waits[1:]:
                    extra = nc.sync.drain()
                    si = extra.ins.sync_info
                    if si is None:
                        import bass_rust
                        extra.ins.sync_info = bass_rust.SyncInfo(
                            on_wait=[w], on_update=[]
                        )
                    else:
                        si.on_wait = [w]
            nc.all_engine_barrier()
            assert self.sems is not None
            popped = nc._tile_sem_poison_stack.pop()
            assert popped is self._sem_poison
            nc.clear_and_free_semaphores(list(self.sems.allocated().values()))
            nc.all_engine_barrier()

        tilemod.TileContext._drain_and_barrier = _drain_and_barrier
        tilemod.TileContext._drain_patch_applied = True

    import antenv
    if not hasattr(antenv, "axon_hooks"):
        mod = types.ModuleType("antenv.axon_hooks")
        mod._hook = None
        mod.set_axon_ntff_profile_hook = lambda h: setattr(mod, "_hook", h)
        mod.get_axon_ntff_profile_hook = lambda: mod._hook
        sys.modules["antenv.axon_hooks"] = mod
        antenv.axon_hooks = mod
        try:
            from trn_agent_boot.trn_boot import _ntff_profile_via_ctypes
            hook = _ntff_profile_via_ctypes("/opt/axon/libaxon_pjrt.so")
            if hook is not None:
                mod.set_axon_ntff_profile_hook(hook)
        except Exception:
            pass


def _split_sync_waits(nc, max_waits=1):
    """The public walrus rejects instructions with >2 sync-wait commands.
    Spread overflow waits onto same-engine NOPs inserted just before."""
    import concourse.mybir as mybir

    for f in nc.m.functions:
        for blk in f.blocks:
            insts = blk.instructions
            out = []
            changed = False
            for inst in insts:
                si = getattr(inst, "sync_info", None)
                waits = list(si.on_wait) if si is not None else []
                if len(waits) > max_waits:
                    changed = True
                    overflow = waits[:-max_waits]
                    si.on_wait = waits[-max_waits:]
                    for i in range(0, len(overflow), max_waits):
                        nop = mybir.InstNoOp(
                            name=nc.get_next_instruction_name(),
                            sync_info=mybir.SyncInfo(
                                on_wait=overflow[i:i + max_waits], on_update=[]
                            ),
                            bass_nofuse=True,
                            engine=inst.engine,
                        )
                        out.append(nop)
                out.append(inst)
            if changed:
                blk.instructions = out


# ---------------------------------------------------------------------------
# kernel builder
# ---------------------------------------------------------------------------
def build_nc(T=T, Tc=64):
    import concourse.bass as bass
    import concourse.tile as tile
    import concourse.mybir as mybir

    f32 = mybir.dt.float32
    f16 = mybir.dt.float16
    NC = T // Tc               # number of chunks
    S = 4 * B                  # free cols per step (4 U-tiles x B)

    nc = bass.Bass(trn_type="TRN2")

    xT_d = nc.declare_dram_parameter("xT", (I, T * B), f16, isOutput=False)
    k0_d = nc.declare_dram_parameter("k0", (I, U), f16, isOutput=False)
    w0_d = nc.declare_dram_parameter("w0", (128, 4 * U), f16, isOutput=False)
    k1_d = nc.declare_dram_parameter("k1", (128, 4 * U), f16, isOutput=False)
    w1_d = nc.declare_dram_parameter("w1", (128, 4 * U), f16, isOutput=False)
    b0_d = nc.declare_dram_parameter("b0", (128, 4), f32, isOutput=False)
    b1_d = nc.declare_dram_parameter("b1", (128, 4), f32, isOutput=False)
    id_d = nc.declare_dram_parameter("ident", (128, 128), f16, isOutput=False)
    h0_d = nc.declare_dram_parameter("h0T", (U, T * B), f16, isOutput=True)
    h1_d = nc.declare_dram_parameter("h1T", (U, T * B), f16, isOutput=True)

    with tile.TileContext(nc) as tc:
        import contextlib
        with contextlib.ExitStack() as ctx:
            const = ctx.enter_context(tc.tile_pool(name="const", bufs=1))
            hists = ctx.enter_context(tc.tile_pool(name="hists", bufs=1))
            upool = ctx.enter_context(tc.tile_pool(name="upool", bufs=1))
            ypool = ctx.enter_context(tc.tile_pool(name="ypool", bufs=3))
            ps_step = ctx.enter_context(
                tc.tile_pool(name="ps_step", bufs=3, space="PSUM"))
            ps_gemm = ctx.enter_context(
                tc.tile_pool(name="ps_gemm", bufs=2, space="PSUM"))

            # --- resident constants -----------------------------------------
            xT = const.tile([I, T * B], f16, tag="xT")
            nc.sync.dma_start(xT[:], xT_d[:, :])
            k0 = const.tile([I, U], f16, tag="k0")
            nc.sync.dma_start(k0[:], k0_d[:, :])
            w0 = const.tile([128, 4 * U], f16, tag="w0")
            nc.sync.dma_start(w0[:], w0_d[:, :])
            k1 = const.tile([128, 4 * U], f16, tag="k1")
            nc.sync.dma_start(k1[:], k1_d[:, :])
            w1 = const.tile([128, 4 * U], f16, tag="w1")
            nc.sync.dma_start(w1[:], w1_d[:, :])
            b0 = const.tile([128, 4], f32, tag="b0")
            nc.sync.dma_start(b0[:], b0_d[:, :])
            b1 = const.tile([128, 4], f32, tag="b1")
            nc.sync.dma_start(b1[:], b1_d[:, :])
            ident = const.tile([128, 128], f16, tag="ident")
            nc.sync.dma_start(ident[:], id_d[:, :])
            zero = const.tile([128, S], f16, tag="zero")
            nc.vector.memset(zero[:], 0.0)

            # double-buffered chunk tensors
            hist0 = [const.tile([128, Tc * S], f16, tag=f"hist0_{i}", name=f"hist0_{i}") for i in range(2)]
            hist1 = [const.tile([128, Tc * S], f16, tag=f"hist1_{i}", name=f"hist1_{i}") for i in range(2)]
            u0c = [const.tile([128, Tc * S], f16, tag=f"u0c_{i}", name=f"u0c_{i}") for i in range(2)]
            u1c = [const.tile([128, Tc * S], f16, tag=f"u1c_{i}", name=f"u1c_{i}") for i in range(2)]

            # chunk buffers use slab layout: col = j*(Tc*B) + t*B + b
            # (j = feature 128-tile). Slabs are contiguous for DMA and GEMMs;
            # per-step access is the strided [128, 4, B] cross-section.
            SLAB = Tc * B

            def mm_rhs_ap(hist_bufs, t, j):
                """[128, B] moving operand: state k-tile j at step t."""
                if t < 0:
                    return zero[:, j * B:(j + 1) * B]
                return hist_bufs[(t // Tc) % 2][
                    :, j * SLAB + (t % Tc) * B: j * SLAB + (t % Tc + 1) * B]

            def state_xs(hist_bufs, t):
                """[128, 4, B] strided cross-section of the state at step t."""
                if t < 0:
                    return zero[:, :].rearrange("p (j b) -> p j b", j=4)
                buf = hist_bufs[(t // Tc) % 2]
                r = t % Tc
                return buf[:, :].rearrange(
                    "p (j tb) -> p j tb", j=4)[:, :, r * B:(r + 1) * B]

            def u0_gemm(c):
                """u0 chunk c = x[chunk]^T-projected, bias added on evacuation."""
                dst = u0c[c % 2]
                cols = slice(c * Tc * B, (c + 1) * Tc * B)
                for m in range(4):
                    pg = ps_gemm.tile([128, Tc * B], mybir.dt.float32, tag="pg", name="pg")
                    nc.tensor.matmul(
                        pg[:], lhsT=k0[:, m * 128:(m + 1) * 128],
                        rhs=xT[:, cols], start=True, stop=True)
                    nc.scalar.activation(
                        dst[:, m * SLAB:(m + 1) * SLAB], pg[:],
                        mybir.ActivationFunctionType.Identity,
                        bias=b0[:, m:m + 1], scale=1.0)

            def u1_gemm(c):
                """u1 chunk c from hist0 chunk c (scales folded into k1)."""
                src = hist0[c % 2]
                dst = u1c[c % 2]
                for m in range(4):
                    pg = ps_gemm.tile([128, Tc * B], mybir.dt.float32, tag="pg", name="pg")
                    for k in range(4):
                        nc.tensor.matmul(
                            pg[:],
                            lhsT=k1[:, k * U + m * 128: k * U + (m + 1) * 128],
                            rhs=src[:, k * SLAB:(k + 1) * SLAB],
                            start=(k == 0), stop=(k == 3))
                    nc.scalar.activation(
                        dst[:, m * SLAB:(m + 1) * SLAB], pg[:],
                        mybir.ActivationFunctionType.Identity,
                        bias=b1[:, m:m + 1], scale=1.0)

            def step(layer, t):
                w, uc, hist = (
                    (w0, u0c, hist0) if layer == 0 else (w1, u1c, hist1))
                r = t % Tc
                u_ap = uc[(t // Tc) % 2][:, :].rearrange(
                    "p (m tb) -> p m tb", m=4)[:, :, r * B:(r + 1) * B]
                zb = ps_step.tile([128, S], mybir.dt.float32, tag="zb", name="zb")
                # Seed PSUM with u_t (I^T @ u = u, start=True sets has_written);
                # depends only on the u chunk, so it runs during the chain gap.
                nc.tensor.matmul(
                    zb[:].rearrange("p (m b) -> p m b", m=4),
                    lhsT=ident[:], rhs=u_ap, start=True, stop=False)
                for m in range(4):
                    for k in range(4):
                        nc.tensor.matmul(
                            zb[:, m * B:(m + 1) * B],
                            lhsT=w[:, k * U + m * 128: k * U + (m + 1) * 128],
                            rhs=mm_rhs_ap(hist, t - 1, k),
                            start=False, stop=(m == 3 and k == 3))
                y = ypool.tile([128, S], f16, tag="y", name="y")
                nc.scalar.activation(
                    y[:], zb[:], mybir.ActivationFunctionType.Tanh)
                nc.vector.scalar_tensor_tensor(
                    state_xs(hist, t), state_xs(hist, t - 1), 0.1,
                    y[:].rearrange("p (j b) -> p j b", j=4),
                    op0=mybir.AluOpType.mult, op1=mybir.AluOpType.add)

            def dma_out(hist_bufs, dram, c):
                src = hist_bufs[c % 2]
                cols = slice(c * Tc * B, (c + 1) * Tc * B)
                for j in range(4):
                    nc.sync.dma_start(
                        dram[j * 128:(j + 1) * 128, cols],
                        src[:, j * SLAB:(j + 1) * SLAB])

            u0_gemm(0)
            for c in range(NC + 1):
                for r in range(Tc):
                    if c < NC:
                        step(0, c * Tc + r)
                    if c >= 1:
                        step(1, (c - 1) * Tc + r)
                if c < NC:
                    u1_gemm(c)
                    dma_out(hist0, h0_d, c)
                    if c + 1 < NC:
                        u0_gemm(c + 1)
                if c >= 1:
                    dma_out(hist1, h1_d, c - 1)

    _split_sync_waits(nc)
    return nc


# ---------------------------------------------------------------------------
# host wrapper
# ---------------------------------------------------------------------------
def _prep_weight(w, scale):
    """[U,U] -> [128, 4*U] fp16 with block (k,m) at cols k*U + m*128."""
    a = (scale * w).astype(np.float16)
    return np.ascontiguousarray(
        a.reshape(4, 128, 4, 128).transpose(1, 0, 2, 3).reshape(128, 4 * U))


def kernel(x, kernel0, rec0, bias0, kernel1, rec1, bias1):
    _apply_patches()
    from concourse.bass_utils import run_bass_kernel_spmd

    x = np.asarray(x, dtype=np.float32)
    kernel0 = np.asarray(kernel0, dtype=np.float32)
    rec0 = np.asarray(rec0, dtype=np.float32)
    bias0 = np.asarray(bias0, dtype=np.float32)
    kernel1 = np.asarray(kernel1, dtype=np.float32)
    rec1 = np.asarray(rec1, dtype=np.float32)
    bias1 = np.asarray(bias1, dtype=np.float32)

    if "nc" not in _COMPILED:
        _COMPILED["nc"] = build_nc()
    nc = _COMPILED["nc"]

    in_maps = _make_in_maps(x, kernel0, rec0, bias0, kernel1, rec1, bias1)
    res = run_bass_kernel_spmd(nc, in_maps, list(range(NCORES)))

    out = np.empty((B_TOT, T, 2 * U), dtype=np.float32)
    for c in range(NCORES):
        for name, off in (("h0T", 0), ("h1T", U)):
            h = res.results[c][name].astype(np.float32) * ALPHA  # [U, T*B]
            out[c * B:(c + 1) * B, :, off:off + U] = (
                h.reshape(U, T, B).transpose(2, 1, 0))
    return out


def _make_in_maps(x, kernel0, rec0, bias0, kernel1, rec1, bias1):
    k0 = kernel0.astype(np.float16)
    w0 = _prep_weight(rec0, ALPHA)
    k1 = _prep_weight(kernel1, ALPHA)
    w1 = _prep_weight(rec1, ALPHA)
    b0 = np.ascontiguousarray(bias0.reshape(4, 128).T).astype(np.float32)
    b1 = np.ascontiguousarray(bias1.reshape(4, 128).T).astype(np.float32)
    ident = np.eye(128, dtype=np.float16)
    in_maps = []
    for c in range(NCORES):
        xc = x[c * B:(c + 1) * B]
        xT = np.ascontiguousarray(
            xc.transpose(2, 1, 0).reshape(I, T * B)).astype(np.float16)
        in_maps.append({
            "xT": xT, "k0": k0, "w0": w0, "k1": k1, "w1": w1,
            "b0": b0, "b1": b1, "ident": ident,
        })
    return in_maps


def run_timed(x, kernel0, rec0, bias0, kernel1, rec1, bias1, tmpdir=None):
    """Run with NTFF profiling; returns BassKernelResults with exec_time_ns."""
    _apply_patches()
    import tempfile
    if tmpdir is None:
        tmpdir = tempfile.mkdtemp(prefix="dr_trace_")
    from concourse.bass_utils import run_bass_kernel_spmd
    if "nc" not in _COMPILED:
        _COMPILED["nc"] = build_nc()
    in_maps = _make_in_maps(
        np.asarray(x, np.float32), np.asarray(kernel0, np.float32),
        np.asarray(rec0, np.float32), np.asarray(bias0, np.float32),
        np.asarray(kernel1, np.float32), np.asarray(rec1, np.float32),
        np.asarray(bias1, np.float32))
    return run_bass_kernel_spmd(
        _COMPILED["nc"], in_maps, list(range(NCORES)), trace=True,
        tmpdir=tmpdir)

